# revision 15
# baseline (speedup 1.0000x reference)
"""Causal self-attention with RoPE for Trainium2, sharded over 8 NeuronCores.

Sharding: core c handles batch b = c % 4 and head-group g = c // 4 (8 of the 16
heads).  Each core computes qkv projection for its heads, RoPE, causal
attention, and a partial out-projection (its heads' columns of w_out); the host
sums the two head-group partials per batch and folds in all bias terms that
commute with the reduction (b_v and b_out).

Device layouts (all prepared host-side in numpy):
  - q/k are built transposed [d, t] with d de-interleaved per head
    ([even components | odd components]) so RoPE is pure row-block arithmetic,
    and scaled contraction order matches between q and k.
  - v is built in natural [t, d] layout with a ones column appended per head,
    so the att @ v matmul also produces the softmax denominator for free.
  - softmax skips the max-subtraction (scores are O(1) here, exp is safe) and
    normalization is deferred to a single per-(head, chunk) rescale.
"""

import sys

for _p in ("/opt/trn_rl_repo",):
    if _p not in sys.path:
        sys.path.insert(0, _p)

import numpy as np

import concourse.bass as bass
import concourse.mybir as mybir
import concourse.tile as tile
from concourse.bass_utils import run_bass_kernel_spmd
from concourse.vector_clock import ScopedClock

# ---------------------------------------------------------------------------
# Workaround for the pinned walrus build: CTRL-class instructions (Drain/NOP)
# accept a single sync-wait, but Tile's kernel-tail drain aggregates one wait
# per active logical processor.  Spread them over a chain of NOPs instead.
# ---------------------------------------------------------------------------
_MAXW = 1


def _split_drain_and_barrier(self, tick_clock, wait_clock):
    nc = self.nc
    probe = nc.sync.nop(nofuse=True, hint="tail_wait_split")
    wait_clock.add_sem_waits(probe.ins, ScopedClock({None: tick_clock.global_clock}))
    si = probe.ins.sync_info
    waits = list(si.on_wait) if si is not None else []
    if si is not None:
        si.on_wait = waits[:_MAXW]
    rest = waits[_MAXW:]
    while rest:
        n = nc.sync.nop(nofuse=True, hint="tail_wait_split")
        n.ins.sync_info = mybir.SyncInfo(on_wait=rest[:_MAXW], on_update=[])
        rest = rest[_MAXW:]
    nc.sync.drain()
    nc.all_engine_barrier()
    assert self.sems is not None
    popped = nc._tile_sem_poison_stack.pop()
    assert popped is self._sem_poison
    nc.clear_and_free_semaphores(list(self.sems.allocated().values()))
    nc.all_engine_barrier()


tile.TileContext._drain_and_barrier = _split_drain_and_barrier


def _split_body_waits(nc: bass.Bass) -> None:
    """Walrus build caps engine instructions at one sync-wait.  Move overflow
    waits onto NOPs inserted just before the instruction on the same engine."""
    for fn in nc.m.functions:
        for blk in fn.blocks:
            insts = list(blk.instructions)
            if not any(
                i.sync_info is not None and len(i.sync_info.on_wait) > 1
                for i in insts
            ):
                continue
            new_insts = []
            for inst in insts:
                si = inst.sync_info
                if si is not None and len(si.on_wait) > 1:
                    waits = list(si.on_wait)
                    for w in waits[:-1]:
                        new_insts.append(
                            mybir.InstNoOp(
                                name=nc.get_next_instruction_name(),
                                engine=inst.engine,
                                ins=[],
                                outs=[],
                                sync_info=mybir.SyncInfo(on_wait=[w], on_update=[]),
                                bass_nofuse=True,
                            )
                        )
                    inst.sync_info = mybir.SyncInfo(
                        on_wait=[waits[-1]], on_update=list(si.on_update)
                    )
                new_insts.append(inst)
            blk.instructions = new_insts

# ---------------------------------------------------------------------------

F32 = mybir.dt.float32
AF = mybir.ActivationFunctionType

B, C = 4, 1024
H, D = 16, 64
HL = H // 2  # heads per core: 8
DH = HL * D  # 512
KO = C // 128  # 8 contraction tiles for the projections
NMT = DH // 128  # 4 M-tiles of q/k (2 heads each)


def build_nc(T: int) -> bass.Bass:
    CT = T // 512  # projection t-chunks
    KT = T // 128  # key tiles
    NQ = T // 512  # query chunks

    nc = bass.Bass()
    xt_d = nc.dram_tensor("xt", [CT, 128, KO, 512], F32, kind="ExternalInput")
    wq_d = nc.dram_tensor("wq", [128, KO, DH], F32, kind="ExternalInput")
    wk_d = nc.dram_tensor("wk", [128, KO, DH], F32, kind="ExternalInput")
    wv_d = nc.dram_tensor("wv", [128, KO, DH], F32, kind="ExternalInput")
    bq_d = nc.dram_tensor("bq", [128, NMT], F32, kind="ExternalInput")
    bk_d = nc.dram_tensor("bk", [128, NMT], F32, kind="ExternalInput")
    cs_d = nc.dram_tensor("cs", [128, T], F32, kind="ExternalInput")
    sn_d = nc.dram_tensor("sn", [128, T], F32, kind="ExternalInput")
    mk_d = nc.dram_tensor("mk", [128, 4, 512], F32, kind="ExternalInput")
    wo_d = nc.dram_tensor("wo", [128, NMT, C], F32, kind="ExternalInput")
    out_d = nc.dram_tensor("out", [T, C], F32, kind="ExternalOutput")

    with tile.TileContext(nc) as tc:
        with tc.tile_pool(name="persist", bufs=1) as P:
            qT = P.tile([128, NMT, T], F32)
            kT = P.tile([128, NMT, T], F32)
            vst = P.tile([128, KT, HL * 65], F32)
            bq_s = P.tile([128, NMT], F32)
            bk_s = P.tile([128, NMT], F32)
            ones64 = P.tile([1, 64], F32)
            nc.sync.dma_start(out=bq_s[:], in_=bq_d[:, :])
            nc.sync.dma_start(out=bk_s[:], in_=bk_d[:, :])
            nc.vector.memset(ones64[:], 1.0)
            vst4 = vst.rearrange("p k (h c) -> p k h c", c=65)
            nc.vector.memset(vst4[:, :, :, 64:65], 1.0)

            # ---------------- Phase A: qkv projections ----------------
            with (
                tc.tile_pool(name="wpool", bufs=1) as W,
                tc.tile_pool(name="xc", bufs=2) as XC,
                tc.tile_pool(name="psA", bufs=4, space="PSUM") as PSA,
            ):
                wq_s = W.tile([128, KO, DH], F32)
                wk_s = W.tile([128, KO, DH], F32)
                wv_s = W.tile([128, KO, DH], F32)
                nc.sync.dma_start(out=wq_s[:], in_=wq_d[:, :, :])
                nc.sync.dma_start(out=wk_s[:], in_=wk_d[:, :, :])
                nc.sync.dma_start(out=wv_s[:], in_=wv_d[:, :, :])

                for ct in range(CT):
                    xc_t = XC.tile([128, KO, 512], F32)
                    nc.sync.dma_start(out=xc_t[:], in_=xt_d[ct, :, :, :])
                    tsl = slice(ct * 512, (ct + 1) * 512)
                    for w_s, b_s, dst in ((wq_s, bq_s, qT), (wk_s, bk_s, kT)):
                        for mt in range(NMT):
                            ps = PSA.tile([128, 512], F32, tag="psA")
                            for ko in range(KO):
                                nc.tensor.matmul(
                                    ps[:],
                                    lhsT=w_s[:, ko, mt * 128 : (mt + 1) * 128],
                                    rhs=xc_t[:, ko, :],
                                    start=(ko == 0),
                                    stop=(ko == KO - 1),
                                )
                            nc.scalar.activation(
                                out=dst[:, mt, tsl],
                                in_=ps[:],
                                func=AF.Identity,
                                bias=b_s[:, mt : mt + 1],
                            )
                    for tt in range(4):
                        ps = PSA.tile([128, 512], F32, tag="psA")
                        for ko in range(KO):
                            nc.tensor.matmul(
                                ps[:],
                                lhsT=xc_t[:, ko, tt * 128 : (tt + 1) * 128],
                                rhs=wv_s[:, ko, :],
                                start=(ko == 0),
                                stop=(ko == KO - 1),
                            )
                        kt_i = ct * 4 + tt
                        nc.vector.tensor_copy(
                            out=vst4[:, kt_i, :, 0:64],
                            in_=ps.rearrange("p (h c) -> p h c", c=64),
                        )

            # ---------------- Phase B: RoPE on q and k ----------------
            # sn holds [-sin | +sin] per 64-block, so with a partition-swapped
            # copy (ev<->od 32-blocks, built by SBUF->SBUF DMA) the rotation is
            # three full-width, base-aligned DVE ops per tile:
            #   rot = src*cs + swap(src)*sn
            with (
                tc.tile_pool(name="rope_cs", bufs=1) as RCS,
                tc.tile_pool(name="rope_tmp", bufs=3) as RT,
                tc.tile_pool(name="rope_sw", bufs=2) as RSW,
            ):
                cs_s = RCS.tile([128, T], F32)
                sn_s = RCS.tile([128, T], F32)
                nc.sync.dma_start(out=cs_s[:], in_=cs_d[:, :])
                nc.sync.dma_start(out=sn_s[:], in_=sn_d[:, :])
                for src in (qT, kT):
                    for mt in range(NMT):
                        sw = RSW.tile([128, T], F32, tag="ropesw")
                        for hb in (0, 64):
                            ev, od = hb, hb + 32
                            nc.sync.dma_start(
                                out=sw[ev : ev + 32, :], in_=src[od : od + 32, mt, :]
                            )
                            nc.sync.dma_start(
                                out=sw[od : od + 32, :], in_=src[ev : ev + 32, mt, :]
                            )
                        a_t = RT.tile([128, T], F32, tag="ropeA")
                        nc.vector.tensor_mul(a_t[:], src[:, mt, :], cs_s[:])
                        nc.vector.tensor_mul(sw[:], sw[:], sn_s[:])
                        nc.vector.tensor_add(src[:, mt, :], a_t[:], sw[:])

            # ---------------- Phases C+D: attention + out-projection ----
            with (
                tc.tile_pool(name="mk", bufs=1) as MK,
                tc.tile_pool(name="att", bufs=8) as AT,
                tc.tile_pool(name="rc", bufs=4) as RC,
                tc.tile_pool(name="ybuf", bufs=1) as YB,
                tc.tile_pool(name="wob", bufs=1) as WO,
                tc.tile_pool(name="ob", bufs=3) as OB,
                tc.tile_pool(name="psG", bufs=4, space="PSUM") as PSG,
                tc.tile_pool(name="psY", bufs=3, space="PSUM") as PSY,
            ):
                mk_s = MK.tile([128, 4, 512], F32)
                nc.sync.dma_start(out=mk_s[:], in_=mk_d[:, :, :])
                yst = YB.tile([128, NMT, T], F32)
                wo_s = WO.tile([128, NMT, C], F32)
                nc.sync.dma_start(out=wo_s[:], in_=wo_d[:, :, :])

                for hp in range(NMT):
                    for bq in range(NQ):
                        qsl = slice(bq * 512, (bq + 1) * 512)
                        jmax = 4 * (bq + 1)
                        # Both heads of the pair interleaved per key tile: their
                        # K=64 score matmuls sit in disjoint PE row groups
                        # (partitions 0-63 vs 64-127) and run concurrently.
                        yps = [
                            PSY.tile([128, 512], F32, tag="psY", name=f"yps{_h}")
                            for _h in range(2)
                        ]
                        for j in range(jmax):
                            ats = []
                            for h2 in (0, 1):
                                off = h2 * 64
                                sps = PSG.tile([128, 512], F32, tag="psG")
                                nc.tensor.matmul(
                                    sps[:],
                                    lhsT=kT[
                                        off : off + 64, hp, j * 128 : (j + 1) * 128
                                    ],
                                    rhs=qT[off : off + 64, hp, qsl],
                                    start=True,
                                    stop=True,
                                )
                                at = AT.tile([128, 512], F32, tag="att")
                                nc.scalar.activation(
                                    out=at[:], in_=sps[:], func=AF.Exp, scale=0.125
                                )
                                if j >= 4 * bq:
                                    nc.vector.tensor_mul(
                                        at[:], at[:], mk_s[:, j - 4 * bq, :]
                                    )
                                ats.append(at)
                            for h2 in (0, 1):
                                h = hp * 2 + h2
                                nc.tensor.matmul(
                                    yps[h2][0:65, :],
                                    lhsT=vst[:, j, h * 65 : (h + 1) * 65],
                                    rhs=ats[h2][:],
                                    start=(j == 0),
                                    stop=(j == jmax - 1),
                                )
                        for h2 in (0, 1):
                            off = h2 * 64
                            rc = RC.tile([1, 512], F32, tag="rc")
                            nc.vector.reciprocal(rc[:], yps[h2][64:65, :])
                            rb = PSG.tile([128, 512], F32, tag="psG")
                            nc.tensor.matmul(
                                rb[0:64, :],
                                lhsT=ones64[:],
                                rhs=rc[:],
                                start=True,
                                stop=True,
                            )
                            rbs = RC.tile([64, 512], F32, tag="rbs")
                            nc.scalar.copy(out=rbs[:], in_=rb[0:64, :])
                            nc.vector.tensor_mul(
                                out=yst[off : off + 64, hp, qsl],
                                in0=yps[h2][0:64, :],
                                in1=rbs[:],
                            )

                for ti in range(KT):
                    for ec in range(C // 512):
                        ops = PSG.tile([128, 512], F32, tag="psG")
                        for p2 in range(NMT):
                            nc.tensor.matmul(
                                ops[:],
                                lhsT=yst[:, p2, ti * 128 : (ti + 1) * 128],
                                rhs=wo_s[:, p2, ec * 512 : (ec + 1) * 512],
                                start=(p2 == 0),
                                stop=(p2 == NMT - 1),
                            )
                        ob = OB.tile([128, 512], F32, tag="ob")
                        nc.vector.tensor_copy(out=ob[:], in_=ops[:])
                        nc.sync.dma_start(
                            out=out_d[ti * 128 : (ti + 1) * 128, ec * 512 : (ec + 1) * 512],
                            in_=ob[:],
                        )
    _split_body_waits(nc)
    return nc


# ---------------------------------------------------------------------------
# Host-side input preparation
# ---------------------------------------------------------------------------


def _qkv_row(h: int, which: int, d) -> np.ndarray:
    # reference reshapes qkv to (..., H, 3*D): channel = h*3D + which*D + d
    return h * 3 * D + which * D + d


def _qk_rows(g: int, which: int) -> np.ndarray:
    rows = []
    for mt in range(NMT):
        for h2 in range(2):
            hg = g * HL + 2 * mt + h2
            rows.extend(_qkv_row(hg, which, np.arange(0, D, 2)))
            rows.extend(_qkv_row(hg, which, np.arange(1, D, 2)))
    return np.asarray(rows)


def _v_rows(g: int) -> np.ndarray:
    return np.concatenate(
        [_qkv_row(g * HL + hl, 2, np.arange(D)) for hl in range(HL)]
    )


def _prep_core(x, w_qkv, b_qkv, w_out, g: int, b: int, T: int):
    CT = T // 512
    rq = _qk_rows(g, 0)
    rk = _qk_rows(g, 1)

    def as_lhsT(w):  # [512, C] -> [128, KO, 512]
        return np.ascontiguousarray(
            w.T.reshape(KO, 128, DH).transpose(1, 0, 2)
        )

    wq = as_lhsT(w_qkv[rq])
    wk = as_lhsT(w_qkv[rk])
    wv = as_lhsT(w_qkv[_v_rows(g)])
    bq = np.ascontiguousarray(b_qkv[rq].reshape(NMT, 128).T)
    bk = np.ascontiguousarray(b_qkv[rk].reshape(NMT, 128).T)
    xt = np.ascontiguousarray(
        x[b].T.reshape(KO, 128, CT, 512).transpose(2, 1, 0, 3)
    )
    theta = (10000.0 ** (-np.arange(0, D, 2, dtype=np.float64) / D)).astype(np.float64)
    ang = np.arange(T, dtype=np.float64)[:, None] * theta[None, :]
    cs = np.tile(np.cos(ang).T.astype(np.float32), (4, 1))
    # signed sin: even 32-blocks get -sin (they receive the odd component),
    # odd 32-blocks get +sin (they receive the even component)
    sn_half = np.sin(ang).T.astype(np.float32)
    sn = np.concatenate([-sn_half, sn_half, -sn_half, sn_half], axis=0)
    pp, jj, ff = np.meshgrid(
        np.arange(128), np.arange(4), np.arange(512), indexing="ij"
    )
    mk = (128 * jj + pp <= ff).astype(np.float32)
    wo = np.ascontiguousarray(
        w_out[:, g * DH : (g + 1) * DH].T.reshape(NMT, 128, C).transpose(1, 0, 2)
    )
    return {
        "xt": xt,
        "wq": wq,
        "wk": wk,
        "wv": wv,
        "bq": bq,
        "bk": bk,
        "cs": cs,
        "sn": sn,
        "mk": mk,
        "wo": wo,
    }


_NC_CACHE: dict = {}


def run_cores(x, w_qkv, b_qkv, w_out, b_out, T: int, **run_kwargs):
    """Run the 8-core kernel; returns full [B, T, C] output plus run results."""
    if T not in _NC_CACHE:
        _NC_CACHE[T] = build_nc(T)
    nc = _NC_CACHE[T]
    in_maps = []
    for c in range(8):
        g, b = c // 4, c % 4
        in_maps.append(_prep_core(x, w_qkv, b_qkv, w_out, g, b, T))
    res = run_bass_kernel_spmd(nc, in_maps, core_ids=list(range(8)), **run_kwargs)
    bias = b_out.astype(np.float64).copy()
    for g in range(2):
        bv = b_qkv[_v_rows(g)]
        bias += w_out[:, g * DH : (g + 1) * DH].astype(np.float64) @ bv.astype(
            np.float64
        )
    out = np.empty((B, T, C), np.float32)
    for b in range(B):
        out[b] = (
            res.results[b]["out"].astype(np.float64)
            + res.results[4 + b]["out"].astype(np.float64)
            + bias[None, :]
        ).astype(np.float32)
    return out, res


def kernel(x, w_qkv, b_qkv, w_out, b_out):
    x = np.asarray(x, np.float32)
    w_qkv = np.asarray(w_qkv, np.float32)
    b_qkv = np.asarray(b_qkv, np.float32)
    w_out = np.asarray(w_out, np.float32)
    b_out = np.asarray(b_out, np.float32)
    T = x.shape[1]
    out, _ = run_cores(x, w_qkv, b_qkv, w_out, b_out, T)
    return out
